# revision 19
# baseline (speedup 1.0000x reference)
"""Min-Euclidean-distance retrieval kernel for Trainium2 (8 NeuronCores).

Reference computation:
    x: [1, 2048, 512], y: [1, 65536, 512] (fp32)
    sq[p, r] = ||x_p||^2 + ||y_r||^2 - 2 <x_p, y_r>
    out = min over (p, r) of sqrt(max(sq, 0))

Sharding: the candidate pool (R) is split across 8 cores, 8192 candidates
each. The host pre-arranges both GEMM operands partition-major in fp8 so
each DMA moves long contiguous per-partition runs and the contraction dim
(d) lands on SBUF partitions with no on-chip transposes.

Per core: 64 stationary candidate tiles (128 cands each), moving operand
streams all 2048 queries through a [128, 2048] fp32 PSUM tile per cand
tile (8 fp8 DoubleRow matmuls, 216ns each on HW — the PE streaming
floor is 512 MMs x 216ns = 110.6us/core). The host folds the -2 scale
into the y operand (exact in fp8), so psum = ||shifted bias|| - 2G needs
only a +y2 bias.

Epilogue is split across engines so neither stalls the PE stream
(ACT-only draining costs 126us > the PE floor):
  A-path (48 tiles): ScalarE h = psum + y2[cand]  (per-partition bias),
                     VectorE acc = min(acc, h)    (bf16, 2x mode)
  V-path (16 tiles): one fused VectorE scalar_tensor_tensor:
                     acc = (psum + y2[cand]) min acc
ACT ~94us, DVE ~95us, both under the PE floor.

The per-query ||x_p||^2 term is constant across candidates, so it is
added on the host, along with the final min across lanes/cores and the
(monotone) sqrt. y2 is shifted by -128 so values near the global min sit
near zero where bf16 rounding is finest.
"""

import sys

for _p in ("/opt/trn_rl_repo", "/root/.axon_site/_ro/trn_rl_repo"):
    if _p not in sys.path:
        sys.path.append(_p)

import ml_dtypes
import numpy as np

import concourse.bass as bass
import concourse.mybir as mybir
import concourse.tile as tile
from concourse import bacc, bass_utils

P = 2048          # queries
R = 65536         # candidates (full)
D = 512           # feature dim
NCORES = 8
R_LOC = R // NCORES      # 8192 candidates per core
P_CHUNKS = P // 512      # 4 moving chunks of queries
R_TILES = R_LOC // 128   # 64 stationary tiles of candidates
R_GROUPS = 16            # y DMA/SBUF grouping: 512 candidates per group
K_TILES = D // 128       # 4 contraction tiles (2 DoubleRow pairs)

F32 = mybir.dt.float32
BF16 = mybir.dt.bfloat16
MM_DT = mybir.dt.float8e4
MM_NP = ml_dtypes.float8_e4m3
ACC_NP = ml_dtypes.bfloat16
# h = (y2 - SHIFT) - 2G; SHIFT centers near-minimum h values around zero
# (sq_min ~ 650, x2 ~ 512 => h_min ~ 650 - x2 - 128 in [-120, +130]) where
# the bf16 quantum is <= 1, so min-chain rounding error is ~2e-4 relative
# on the final distance. min-accumulation itself is exact in any format.
Y2_SHIFT = np.float32(128.0)
# x2 is shifted to center the bf16 broadcast row near zero for the final
# on-device (acc + x2) min-reduce.
X2_SHIFT = np.float32(512.0)


def _build_module() -> bass.Bass:
    nc = bacc.Bacc("TRN2", target_bir_lowering=False, debug=False)

    # Host-prepared layouts (partition-major, contiguous per partition):
    #   xt[q, c, k, j] = x[c*512 + j, k*128 + q]          (fp8)
    #   yt[q, g, k, s] = -2 * y[g*512 + s, k*128 + q]     (fp8, -2 folded in)
    #   y2t[lane, t]   = ||y_r||^2 - 128 for r = t*128 + lane  (fp32)
    xt = nc.dram_tensor("xt", [128, P_CHUNKS, K_TILES, 512], MM_DT,
                        kind="ExternalInput")
    yt = nc.dram_tensor("yt", [128, R_GROUPS, K_TILES, 512], MM_DT,
                        kind="ExternalInput")
    y2t = nc.dram_tensor("y2t", [128, R_TILES], F32, kind="ExternalInput")
    #   x2r[q, p] = ||x_p||^2 - 512  (bf16, replicated rows, all cores)
    x2r = nc.dram_tensor("x2r", [128, P], BF16, kind="ExternalInput")
    # out[lane, 0] = min over p of (acc[lane, p] + x2b[p]) = min_p sq - 640
    out = nc.dram_tensor("out", [128, 1], F32, kind="ExternalOutput")

    with tile.TileContext(nc) as tc:
        with (
            tc.tile_pool(name="big", bufs=1) as big,
            tc.tile_pool(name="scr", bufs=4) as scr,
            tc.tile_pool(name="psum", bufs=4, space="PSUM") as psum,
        ):
            xt_sb = big.tile([128, P_CHUNKS, K_TILES, 512], MM_DT)
            yt_sb = big.tile([128, R_GROUPS, K_TILES, 512], MM_DT)
            y2t_sb = big.tile([128, R_TILES], F32)
            acc = big.tile([128, P], BF16)
            x2b = big.tile([128, P], BF16)
            red_out = big.tile([128, 1], F32)
            ttr_scratch = big.tile([128, P], BF16)
            wz = big.tile([128, 2, 128], MM_DT)
            wx = big.tile([128, 2, 512], MM_DT)

            # DMA plan: only 3 queues exist (sync/SP, scalar/Activation,
            # gpsimd SWDGE), each an independent ~47 GB/s ring. x (1MB) is
            # split into 8 half-chunk slices over scalar+gpsimd so it is
            # fully resident by ~19us (vs ~28us on one queue — the first
            # ~10 candidate tiles were DMA-starved in v1). y (4MB) streams
            # on the sync ring in 32 (group, kk) slices: steady consumption
            # is 128KB per 1.73us < one ring's rate, with growing slack.
            # y2t goes first on gpsimd (needed by the first drain, tiny).
            def x_slice(c, kk):
                return (xt_sb[:, c, 2 * kk : 2 * kk + 2, :],
                        xt.ap()[:, c, 2 * kk : 2 * kk + 2, :])

            # PE warm-up: the HAM clock gate holds the PE at 1.2 GHz until
            # it has been busy ~3.4us. Eight dummy DoubleRow matmuls on
            # zeroed tiles burn exactly that window before the first real
            # matmul's data lands, so the real stream starts at 2.4 GHz
            # (v5 ran its first ~12 matmuls at half clock).
            nc.gpsimd.memset(wz[:], 0.0)
            nc.gpsimd.memset(wx[:], 0.0)
            nc.gpsimd.dma_start(y2t_sb[:], y2t.ap())
            for g in range(R_GROUPS):
                for kk in range(K_TILES // 2):
                    nc.sync.dma_start(
                        yt_sb[:, g, 2 * kk : 2 * kk + 2, :],
                        yt.ap()[:, g, 2 * kk : 2 * kk + 2, :],
                    )
            for c in range(P_CHUNKS):
                nc.scalar.dma_start(*x_slice(c, 0))
                nc.gpsimd.dma_start(*x_slice(c, 1))
            # x2 (host-replicated across partitions) for the on-device
            # final reduce; lands during the long idle stretch of the
            # gpsimd ring. GpSimd issues nothing after this, so its
            # expensive end-of-kernel SWDGE drain (4.9us, which sat on the
            # critical path after the v5 output DMA) runs mid-kernel,
            # fully overlapped.
            nc.gpsimd.dma_start(x2b[:], x2r.ap())

            WARMUP_MMS = 0
            if WARMUP_MMS:
                wp = psum.tile([128, P // 2], F32, name="pt")
                for _ in range(WARMUP_MMS):
                    nc.tensor.matmul(
                        wp[:, 0:512],
                        lhsT=wz[:],
                        rhs=wx[:],
                        start=True,
                        stop=True,
                        perf_mode=mybir.MatmulPerfMode.DoubleRow,
                    )

            # Each candidate tile is processed as two half-width PSUM tiles
            # ([128, 1024] = 2 banks, bufs=4 = all 8 banks). A tile's PSUM
            # release op (ACT 1965ns / STT 2350ns at full width) exceeded
            # the 1728ns tile period, so with bufs=2 the PE stalled on
            # every release (measured 20us). At half width the release is
            # ~1.2-1.5us against a 2.6us three-buffer tolerance.
            #
            # Drain paths per half H = 2t+hf (engine balance vs the 110.6us
            # PE floor, measured op costs ACT 1110ns / STT 1278ns / TT
            # 688ns): V-halves (every 4th, 32x) use the fused DVE STT; the
            # rest use ACT h=psum+y2 then a DVE min. ACT ~106us, DVE
            # ~106us. (GpSimd cannot run TENSOR_TENSOR on TRN2.)
            #
            # A-halves' min ops are emitted with one half of lag so a
            # V-half's fused STT can run the moment its matmuls stop
            # (min-accumulation commutes, so chain order is free): the STT
            # would otherwise wait on ACT(H-1)+min(H-1) and free its PSUM
            # late.
            pending_min = []

            def flush_mins():
                while pending_min:
                    eng, acc_, h_ = pending_min.pop(0)
                    eng.tensor_tensor(
                        out=acc_, in0=acc_, in1=h_, op=mybir.AluOpType.min
                    )

            HP = P // 2  # half width
            for t in range(R_TILES):
                g, o = t // 4, (t % 4) * 128
                bias = y2t_sb[:, t : t + 1]
                for hf in range(2):
                    H = 2 * t + hf
                    acc_h = acc[:, hf * HP : (hf + 1) * HP]
                    pt = psum.tile([128, HP], F32, name="pt")
                    for c in (2 * hf, 2 * hf + 1):
                        for kk in range(K_TILES // 2):
                            nc.tensor.matmul(
                                pt[:, (c - 2 * hf) * 512 : (c - 2 * hf + 1) * 512],
                                lhsT=yt_sb[:, g, 2 * kk : 2 * kk + 2, o : o + 128],
                                rhs=xt_sb[:, c, 2 * kk : 2 * kk + 2, :],
                                start=(kk == 0),
                                stop=(kk == K_TILES // 2 - 1),
                                perf_mode=mybir.MatmulPerfMode.DoubleRow,
                            )
                    if H < 2:
                        # First drain of each acc column half initializes it.
                        nc.scalar.activation(
                            out=acc_h,
                            in_=pt[:],
                            func=mybir.ActivationFunctionType.Identity,
                            bias=bias,
                            scale=1.0,
                        )
                    elif H % 4 == 2 or H >= 126:
                        # Fused drain: acc = min(psum + y2, acc) in one DVE op.
                        nc.vector.scalar_tensor_tensor(
                            out=acc_h,
                            in0=pt[:],
                            scalar=bias,
                            in1=acc_h,
                            op0=mybir.AluOpType.add,
                            op1=mybir.AluOpType.min,
                        )
                        flush_mins()
                    else:
                        h = scr.tile([128, HP], BF16, name="h")
                        nc.scalar.activation(
                            out=h[:],
                            in_=pt[:],
                            func=mybir.ActivationFunctionType.Identity,
                            bias=bias,
                            scale=1.0,
                        )
                        flush_mins()
                        pending_min.append((nc.vector, acc_h, h[:]))
            flush_mins()
            # Fused final reduce: red[lane] = min over p of (acc + x2b),
            # then ship 4 bytes/partition instead of 4KB (the 512KB ship
            # plus GpSimd's trailing drain cost v5 ~8us of tail).
            nc.vector.tensor_tensor(
                out=ttr_scratch[:],
                in0=acc[:],
                in1=x2b[:],
                op=mybir.AluOpType.add,
            )
            nc.vector.tensor_reduce(
                out=red_out[:],
                in_=ttr_scratch[:],
                axis=mybir.AxisListType.XY,
                op=mybir.AluOpType.min,
            )
            nc.sync.dma_start(out.ap(), red_out[:])
    nc.compile()
    return nc


_module_cache: bass.Bass | None = None


def _get_module() -> bass.Bass:
    global _module_cache
    if _module_cache is None:
        _module_cache = _build_module()
    return _module_cache


def _to_partition_major(at: np.ndarray, nchunks: int) -> np.ndarray:
    """[D, W] transposed operand -> [128, nchunks, K_TILES, 512] fp8."""
    w = at.shape[1]
    a4 = at.reshape(K_TILES, 128, nchunks, w // nchunks)
    return np.ascontiguousarray(a4.transpose(1, 2, 0, 3).astype(MM_NP))


def _prepare_inputs(x: np.ndarray, y: np.ndarray):
    """Host-side sharding/layout prep. Returns per-core input maps."""
    xt = _to_partition_major(x.T, P_CHUNKS)
    x2 = np.einsum("pd,pd->p", x, x, dtype=np.float32) - X2_SHIFT
    x2r = np.ascontiguousarray(
        np.broadcast_to(x2.astype(ACC_NP), (128, P))
    )
    in_maps = []
    for c in range(NCORES):
        yc = y[c * R_LOC : (c + 1) * R_LOC]
        # -2 folded into the stationary operand: exact in fp8 (sign+exponent)
        yct = _to_partition_major(np.float32(-2.0) * yc.T, R_GROUPS)
        y2 = np.einsum("rd,rd->r", yc, yc, dtype=np.float32) - Y2_SHIFT
        y2t = np.ascontiguousarray(y2.reshape(R_TILES, 128).T)
        in_maps.append({"xt": xt, "yt": yct, "y2t": y2t, "x2r": x2r})
    return in_maps


def _postprocess(accs: np.ndarray) -> np.ndarray:
    """accs: [NCORES, 128, 1] per-lane mins of (sq - Y2_SHIFT - X2_SHIFT)."""
    sq_min = np.float32(accs.astype(np.float32).min() + Y2_SHIFT + X2_SHIFT)
    return np.sqrt(np.maximum(sq_min, np.float32(0.0)), dtype=np.float32)


def kernel(
    predicted_transaction_company: np.ndarray,
    future_transaction_companies_inc_current_data: np.ndarray,
) -> np.ndarray:
    x = np.asarray(predicted_transaction_company, dtype=np.float32)[0]
    y = np.asarray(future_transaction_companies_inc_current_data, dtype=np.float32)[0]

    nc = _get_module()
    in_maps = _prepare_inputs(x, y)
    res = bass_utils.run_bass_kernel_spmd(nc, in_maps, core_ids=list(range(NCORES)))
    accs = np.stack([r["out"] for r in res.results])
    return _postprocess(accs)


# revision 20
# speedup vs baseline: 1.0272x; 1.0272x over previous
"""Min-Euclidean-distance retrieval kernel for Trainium2 (8 NeuronCores).

Reference computation:
    x: [1, 2048, 512], y: [1, 65536, 512] (fp32)
    sq[p, r] = ||x_p||^2 + ||y_r||^2 - 2 <x_p, y_r>
    out = min over (p, r) of sqrt(max(sq, 0))

Sharding: the candidate pool (R) is split across 8 cores, 8192 candidates
each. The host pre-arranges both GEMM operands partition-major in fp8 so
each DMA moves long contiguous per-partition runs and the contraction dim
(d) lands on SBUF partitions with no on-chip transposes.

Per core: 64 stationary candidate tiles (128 cands each), moving operand
streams all 2048 queries through a [128, 2048] fp32 PSUM tile per cand
tile (8 fp8 DoubleRow matmuls, 216ns each on HW — the PE streaming
floor is 512 MMs x 216ns = 110.6us/core). The host folds the -2 scale
into the y operand (exact in fp8), so psum = ||shifted bias|| - 2G needs
only a +y2 bias.

Epilogue is split across engines so neither stalls the PE stream
(ACT-only draining costs 126us > the PE floor):
  A-path (48 tiles): ScalarE h = psum + y2[cand]  (per-partition bias),
                     VectorE acc = min(acc, h)    (bf16, 2x mode)
  V-path (16 tiles): one fused VectorE scalar_tensor_tensor:
                     acc = (psum + y2[cand]) min acc
ACT ~94us, DVE ~95us, both under the PE floor.

The per-query ||x_p||^2 term is constant across candidates, so it is
added on the host, along with the final min across lanes/cores and the
(monotone) sqrt. y2 is shifted by -128 so values near the global min sit
near zero where bf16 rounding is finest.
"""

import sys

for _p in ("/opt/trn_rl_repo", "/root/.axon_site/_ro/trn_rl_repo"):
    if _p not in sys.path:
        sys.path.append(_p)

import ml_dtypes
import numpy as np

import concourse.bass as bass
import concourse.mybir as mybir
import concourse.tile as tile
from concourse import bacc, bass_utils

P = 2048          # queries
R = 65536         # candidates (full)
D = 512           # feature dim
NCORES = 8
R_LOC = R // NCORES      # 8192 candidates per core
P_CHUNKS = P // 512      # 4 moving chunks of queries
R_TILES = R_LOC // 128   # 64 stationary tiles of candidates
R_GROUPS = 16            # y DMA/SBUF grouping: 512 candidates per group
K_TILES = D // 128       # 4 contraction tiles (2 DoubleRow pairs)

F32 = mybir.dt.float32
BF16 = mybir.dt.bfloat16
MM_DT = mybir.dt.float8e4
MM_NP = ml_dtypes.float8_e4m3
ACC_NP = ml_dtypes.bfloat16
# h = (y2 - SHIFT) - 2G; SHIFT centers near-minimum h values around zero
# (sq_min ~ 650, x2 ~ 512 => h_min ~ 650 - x2 - 128 in [-120, +130]) where
# the bf16 quantum is <= 1, so min-chain rounding error is ~2e-4 relative
# on the final distance. min-accumulation itself is exact in any format.
Y2_SHIFT = np.float32(128.0)
# x2 is shifted to center the bf16 broadcast row near zero for the final
# on-device (acc + x2) min-reduce.
X2_SHIFT = np.float32(512.0)


def _build_module() -> bass.Bass:
    nc = bacc.Bacc("TRN2", target_bir_lowering=False, debug=False)

    # Host-prepared layouts (partition-major, contiguous per partition):
    #   xt[q, c, k, j] = x[c*512 + j, k*128 + q]          (fp8)
    #   yt[q, g, k, s] = -2 * y[g*512 + s, k*128 + q]     (fp8, -2 folded in)
    #   y2t[lane, t]   = ||y_r||^2 - 128 for r = t*128 + lane  (fp32)
    xt = nc.dram_tensor("xt", [128, P_CHUNKS, K_TILES, 512], MM_DT,
                        kind="ExternalInput")
    yt = nc.dram_tensor("yt", [128, R_GROUPS, K_TILES, 512], MM_DT,
                        kind="ExternalInput")
    y2t = nc.dram_tensor("y2t", [128, R_TILES], F32, kind="ExternalInput")
    #   x2r[q, p] = ||x_p||^2 - 512  (bf16, replicated rows, all cores)
    x2r = nc.dram_tensor("x2r", [128, P], BF16, kind="ExternalInput")
    # out[lane, 0] = min over p>=1024 of (acc + x2b)[lane, p] = min sq - 640
    out = nc.dram_tensor("out", [128, 1], F32, kind="ExternalOutput")
    # raw hf0 half of acc (host reduces this one)
    out0 = nc.dram_tensor("out0", [128, P // 2], BF16, kind="ExternalOutput")

    with tile.TileContext(nc) as tc:
        with (
            tc.tile_pool(name="big", bufs=1) as big,
            tc.tile_pool(name="scr", bufs=4) as scr,
            tc.tile_pool(name="psum", bufs=4, space="PSUM") as psum,
        ):
            xt_sb = big.tile([128, P_CHUNKS, K_TILES, 512], MM_DT)
            yt_sb = big.tile([128, R_GROUPS, K_TILES, 512], MM_DT)
            y2t_sb = big.tile([128, R_TILES], F32)
            acc = big.tile([128, P], BF16)
            x2b = big.tile([128, P], BF16)
            red_out = big.tile([128, 1], F32)
            ttr_scratch = big.tile([128, P], BF16)
            wz = big.tile([128, 2, 128], MM_DT)
            wx = big.tile([128, 2, 512], MM_DT)

            # DMA plan: only 3 queues exist (sync/SP, scalar/Activation,
            # gpsimd SWDGE), each an independent ~47 GB/s ring. x (1MB) is
            # split into 8 half-chunk slices over scalar+gpsimd so it is
            # fully resident by ~19us (vs ~28us on one queue — the first
            # ~10 candidate tiles were DMA-starved in v1). y (4MB) streams
            # on the sync ring in 32 (group, kk) slices: steady consumption
            # is 128KB per 1.73us < one ring's rate, with growing slack.
            # y2t goes first on gpsimd (needed by the first drain, tiny).
            def x_slice(c, kk):
                return (xt_sb[:, c, 2 * kk : 2 * kk + 2, :],
                        xt.ap()[:, c, 2 * kk : 2 * kk + 2, :])

            # PE warm-up: the HAM clock gate holds the PE at 1.2 GHz until
            # it has been busy ~3.4us. Eight dummy DoubleRow matmuls on
            # zeroed tiles burn exactly that window before the first real
            # matmul's data lands, so the real stream starts at 2.4 GHz
            # (v5 ran its first ~12 matmuls at half clock).
            nc.gpsimd.memset(wz[:], 0.0)
            nc.gpsimd.memset(wx[:], 0.0)
            nc.gpsimd.dma_start(y2t_sb[:], y2t.ap())
            for g in range(R_GROUPS):
                for kk in range(K_TILES // 2):
                    nc.sync.dma_start(
                        yt_sb[:, g, 2 * kk : 2 * kk + 2, :],
                        yt.ap()[:, g, 2 * kk : 2 * kk + 2, :],
                    )
            for c in range(P_CHUNKS):
                nc.scalar.dma_start(*x_slice(c, 0))
                nc.gpsimd.dma_start(*x_slice(c, 1))
            # x2 (host-replicated across partitions) for the on-device
            # final reduce; lands during the long idle stretch of the
            # gpsimd ring. GpSimd issues nothing after this, so its
            # expensive end-of-kernel SWDGE drain (4.9us, which sat on the
            # critical path after the v5 output DMA) runs mid-kernel,
            # fully overlapped.
            nc.gpsimd.dma_start(x2b[:], x2r.ap())

            WARMUP_MMS = 8
            if WARMUP_MMS:
                wp = psum.tile([128, P // 2], F32, name="pt")
                for _ in range(WARMUP_MMS):
                    nc.tensor.matmul(
                        wp[:, 0:512],
                        lhsT=wz[:],
                        rhs=wx[:],
                        start=True,
                        stop=True,
                        perf_mode=mybir.MatmulPerfMode.DoubleRow,
                    )

            # Each candidate tile is processed as two half-width PSUM tiles
            # ([128, 1024] = 2 banks, bufs=4 = all 8 banks). A tile's PSUM
            # release op (ACT 1965ns / STT 2350ns at full width) exceeded
            # the 1728ns tile period, so with bufs=2 the PE stalled on
            # every release (measured 20us). At half width the release is
            # ~1.2-1.5us against a 2.6us three-buffer tolerance.
            #
            # Drain paths per half H = 2t+hf (engine balance vs the 110.6us
            # PE floor, measured op costs ACT 1110ns / STT 1278ns / TT
            # 688ns): V-halves (every 4th, 32x) use the fused DVE STT; the
            # rest use ACT h=psum+y2 then a DVE min. ACT ~106us, DVE
            # ~106us. (GpSimd cannot run TENSOR_TENSOR on TRN2.)
            #
            # A-halves' min ops are emitted with one half of lag so a
            # V-half's fused STT can run the moment its matmuls stop
            # (min-accumulation commutes, so chain order is free): the STT
            # would otherwise wait on ACT(H-1)+min(H-1) and free its PSUM
            # late.
            pending_min = []

            def flush_mins():
                while pending_min:
                    eng, acc_, h_ = pending_min.pop(0)
                    eng.tensor_tensor(
                        out=acc_, in0=acc_, in1=h_, op=mybir.AluOpType.min
                    )

            HP = P // 2  # half width
            for t in range(R_TILES):
                g, o = t // 4, (t % 4) * 128
                bias = y2t_sb[:, t : t + 1]
                for hf in range(2):
                    H = 2 * t + hf
                    acc_h = acc[:, hf * HP : (hf + 1) * HP]
                    pt = psum.tile([128, HP], F32, name="pt")
                    for c in (2 * hf, 2 * hf + 1):
                        for kk in range(K_TILES // 2):
                            nc.tensor.matmul(
                                pt[:, (c - 2 * hf) * 512 : (c - 2 * hf + 1) * 512],
                                lhsT=yt_sb[:, g, 2 * kk : 2 * kk + 2, o : o + 128],
                                rhs=xt_sb[:, c, 2 * kk : 2 * kk + 2, :],
                                start=(kk == 0),
                                stop=(kk == K_TILES // 2 - 1),
                                perf_mode=mybir.MatmulPerfMode.DoubleRow,
                            )
                    if H < 2:
                        # First drain of each acc column half initializes it.
                        nc.scalar.activation(
                            out=acc_h,
                            in_=pt[:],
                            func=mybir.ActivationFunctionType.Identity,
                            bias=bias,
                            scale=1.0,
                        )
                    elif H % 4 == 2 or H >= 126:
                        # Fused drain: acc = min(psum + y2, acc) in one DVE op.
                        nc.vector.scalar_tensor_tensor(
                            out=acc_h,
                            in0=pt[:],
                            scalar=bias,
                            in1=acc_h,
                            op0=mybir.AluOpType.add,
                            op1=mybir.AluOpType.min,
                        )
                        flush_mins()
                    else:
                        h = scr.tile([128, HP], BF16, name="h")
                        nc.scalar.activation(
                            out=h[:],
                            in_=pt[:],
                            func=mybir.ActivationFunctionType.Identity,
                            bias=bias,
                            scale=1.0,
                        )
                        flush_mins()
                        pending_min.append((nc.vector, acc_h, h[:]))
            flush_mins()
            # Hybrid tail: the hf0 column half (final one half-tile early)
            # ships raw on the two HWDGE rings (its reduce happens on the
            # host); hf1 is add+min-reduced on DVE to [128, 1] and shipped
            # via SWDGE (HWDGE crawls ~7us on 4B/partition descriptors).
            nc.sync.dma_start(out0.ap()[:, 0:512], acc[:, 0:512])
            nc.scalar.dma_start(out0.ap()[:, 512:HP], acc[:, 512:HP])
            nc.vector.tensor_tensor(
                out=ttr_scratch[:, 0:HP],
                in0=acc[:, HP:P],
                in1=x2b[:, HP:P],
                op=mybir.AluOpType.add,
            )
            nc.vector.tensor_reduce(
                out=red_out[:],
                in_=ttr_scratch[:, 0:HP],
                axis=mybir.AxisListType.XY,
                op=mybir.AluOpType.min,
            )
            nc.gpsimd.dma_start(out.ap(), red_out[:])
    nc.compile()
    return nc


_module_cache: bass.Bass | None = None


def _get_module() -> bass.Bass:
    global _module_cache
    if _module_cache is None:
        _module_cache = _build_module()
    return _module_cache


def _to_partition_major(at: np.ndarray, nchunks: int) -> np.ndarray:
    """[D, W] transposed operand -> [128, nchunks, K_TILES, 512] fp8."""
    w = at.shape[1]
    a4 = at.reshape(K_TILES, 128, nchunks, w // nchunks)
    return np.ascontiguousarray(a4.transpose(1, 2, 0, 3).astype(MM_NP))


def _prepare_inputs(x: np.ndarray, y: np.ndarray):
    """Host-side sharding/layout prep. Returns per-core input maps."""
    xt = _to_partition_major(x.T, P_CHUNKS)
    x2 = np.einsum("pd,pd->p", x, x, dtype=np.float32) - X2_SHIFT
    x2r = np.ascontiguousarray(
        np.broadcast_to(x2.astype(ACC_NP), (128, P))
    )
    in_maps = []
    for c in range(NCORES):
        yc = y[c * R_LOC : (c + 1) * R_LOC]
        # -2 folded into the stationary operand: exact in fp8 (sign+exponent)
        yct = _to_partition_major(np.float32(-2.0) * yc.T, R_GROUPS)
        y2 = np.einsum("rd,rd->r", yc, yc, dtype=np.float32) - Y2_SHIFT
        y2t = np.ascontiguousarray(y2.reshape(R_TILES, 128).T)
        in_maps.append({"xt": xt, "yt": yct, "y2t": y2t, "x2r": x2r})
    return in_maps


def _postprocess(x: np.ndarray, reds: np.ndarray, acc0s: np.ndarray) -> np.ndarray:
    """reds: [NCORES, 128, 1] device mins of (sq - 640) over p >= 1024;
    acc0s: [NCORES, 128, 1024] raw h for p < 1024 (missing x2)."""
    m1 = np.float32(reds.astype(np.float32).min())
    x2 = np.einsum("pd,pd->p", x, x, dtype=np.float32) - X2_SHIFT
    h0 = acc0s.astype(np.float32).min(axis=(0, 1)) + x2[: P // 2]
    m = min(m1, np.float32(h0.min()))
    sq_min = np.float32(m + Y2_SHIFT + X2_SHIFT)
    return np.sqrt(np.maximum(sq_min, np.float32(0.0)), dtype=np.float32)


def kernel(
    predicted_transaction_company: np.ndarray,
    future_transaction_companies_inc_current_data: np.ndarray,
) -> np.ndarray:
    x = np.asarray(predicted_transaction_company, dtype=np.float32)[0]
    y = np.asarray(future_transaction_companies_inc_current_data, dtype=np.float32)[0]

    nc = _get_module()
    in_maps = _prepare_inputs(x, y)
    res = bass_utils.run_bass_kernel_spmd(nc, in_maps, core_ids=list(range(NCORES)))
    reds = np.stack([r["out"] for r in res.results])
    acc0s = np.stack([r["out0"] for r in res.results])
    return _postprocess(x, reds, acc0s)


# revision 21
# speedup vs baseline: 1.0662x; 1.0380x over previous
"""Min-Euclidean-distance retrieval kernel for Trainium2 (8 NeuronCores).

Reference computation:
    x: [1, 2048, 512], y: [1, 65536, 512] (fp32)
    sq[p, r] = ||x_p||^2 + ||y_r||^2 - 2 <x_p, y_r>
    out = min over (p, r) of sqrt(max(sq, 0))

Sharding: the candidate pool (R) is split across 8 cores, 8192 candidates
each. The host pre-arranges both GEMM operands partition-major in fp8 so
each DMA moves long contiguous per-partition runs and the contraction dim
(d) lands on SBUF partitions with no on-chip transposes.

Per core: 64 stationary candidate tiles (128 cands each), moving operand
streams all 2048 queries through a [128, 2048] fp32 PSUM tile per cand
tile (8 fp8 DoubleRow matmuls, 216ns each on HW — the PE streaming
floor is 512 MMs x 216ns = 110.6us/core). The host folds the -2 scale
into the y operand (exact in fp8), so psum = ||shifted bias|| - 2G needs
only a +y2 bias.

Epilogue is split across engines so neither stalls the PE stream
(ACT-only draining costs 126us > the PE floor):
  A-path (48 tiles): ScalarE h = psum + y2[cand]  (per-partition bias),
                     VectorE acc = min(acc, h)    (bf16, 2x mode)
  V-path (16 tiles): one fused VectorE scalar_tensor_tensor:
                     acc = (psum + y2[cand]) min acc
ACT ~94us, DVE ~95us, both under the PE floor.

The per-query ||x_p||^2 term is constant across candidates, so it is
added on the host, along with the final min across lanes/cores and the
(monotone) sqrt. y2 is shifted by -128 so values near the global min sit
near zero where bf16 rounding is finest.
"""

import sys

for _p in ("/opt/trn_rl_repo", "/root/.axon_site/_ro/trn_rl_repo"):
    if _p not in sys.path:
        sys.path.append(_p)

import ml_dtypes
import numpy as np

import concourse.bass as bass
import concourse.mybir as mybir
import concourse.tile as tile
from concourse import bacc, bass_utils

P = 2048          # queries
R = 65536         # candidates (full)
D = 512           # feature dim
NCORES = 8
R_LOC = R // NCORES      # 8192 candidates per core
P_CHUNKS = P // 512      # 4 moving chunks of queries
R_TILES = R_LOC // 128   # 64 stationary tiles of candidates
R_GROUPS = 16            # y DMA/SBUF grouping: 512 candidates per group
K_TILES = D // 128       # 4 contraction tiles (2 DoubleRow pairs)

F32 = mybir.dt.float32
BF16 = mybir.dt.bfloat16
MM_DT = mybir.dt.float8e4
MM_NP = ml_dtypes.float8_e4m3
ACC_NP = ml_dtypes.bfloat16
# h = (y2 - SHIFT) - 2G; SHIFT centers near-minimum h values around zero
# (sq_min ~ 650, x2 ~ 512 => h_min ~ 650 - x2 - 128 in [-120, +130]) where
# the bf16 quantum is <= 1, so min-chain rounding error is ~2e-4 relative
# on the final distance. min-accumulation itself is exact in any format.
Y2_SHIFT = np.float32(128.0)
# x2 is shifted to center the bf16 broadcast row near zero for the final
# on-device (acc + x2) min-reduce.
X2_SHIFT = np.float32(512.0)


def _build_module() -> bass.Bass:
    nc = bacc.Bacc("TRN2", target_bir_lowering=False, debug=False)

    # Host-prepared layouts (partition-major, contiguous per partition):
    #   xt[q, c, k, j] = x[c*512 + j, k*128 + q]          (fp8)
    #   yt[q, g, k, s] = -2 * y[g*512 + s, k*128 + q]     (fp8, -2 folded in)
    #   y2t[lane, t]   = ||y_r||^2 - 128 for r = t*128 + lane  (fp32)
    xt = nc.dram_tensor("xt", [128, P_CHUNKS, K_TILES, 512], MM_DT,
                        kind="ExternalInput")
    yt = nc.dram_tensor("yt", [128, R_GROUPS, K_TILES, 512], MM_DT,
                        kind="ExternalInput")
    y2t = nc.dram_tensor("y2t", [128, R_TILES], F32, kind="ExternalInput")
    # acc[lane, p] = min over r-tiles t of ((y2[t*128+lane] - 128) - 2 G)
    out = nc.dram_tensor("out", [128, P], BF16, kind="ExternalOutput")

    with tile.TileContext(nc) as tc:
        with (
            tc.tile_pool(name="big", bufs=1) as big,
            tc.tile_pool(name="scr", bufs=4) as scr,
            tc.tile_pool(name="psum", bufs=4, space="PSUM") as psum,
        ):
            xt_sb = big.tile([128, P_CHUNKS, K_TILES, 512], MM_DT)
            yt_sb = big.tile([128, R_GROUPS, K_TILES, 512], MM_DT)
            y2t_sb = big.tile([128, R_TILES], F32)
            acc = big.tile([128, P], BF16)
            wz = big.tile([128, 2, 128], MM_DT)
            wx = big.tile([128, 2, 512], MM_DT)

            # DMA plan: only 3 queues exist (sync/SP, scalar/Activation,
            # gpsimd SWDGE), each an independent ~47 GB/s ring. x (1MB) is
            # split into 8 half-chunk slices over scalar+gpsimd so it is
            # fully resident by ~19us (vs ~28us on one queue — the first
            # ~10 candidate tiles were DMA-starved in v1). y (4MB) streams
            # on the sync ring in 32 (group, kk) slices: steady consumption
            # is 128KB per 1.73us < one ring's rate, with growing slack.
            # y2t goes first on gpsimd (needed by the first drain, tiny).
            def x_slice(c, kk):
                return (xt_sb[:, c, 2 * kk : 2 * kk + 2, :],
                        xt.ap()[:, c, 2 * kk : 2 * kk + 2, :])

            # PE warm-up: the HAM clock gate holds the PE at 1.2 GHz until
            # it has been busy ~3.4us. Eight dummy DoubleRow matmuls on
            # zeroed tiles burn exactly that window before the first real
            # matmul's data lands, so the real stream starts at 2.4 GHz
            # (v5 ran its first ~12 matmuls at half clock).
            nc.gpsimd.memset(wz[:], 0.0)
            nc.gpsimd.memset(wx[:], 0.0)
            nc.gpsimd.dma_start(y2t_sb[:], y2t.ap())
            for g in range(R_GROUPS):
                for kk in range(K_TILES // 2):
                    nc.sync.dma_start(
                        yt_sb[:, g, 2 * kk : 2 * kk + 2, :],
                        yt.ap()[:, g, 2 * kk : 2 * kk + 2, :],
                    )
            for c in range(P_CHUNKS):
                nc.scalar.dma_start(*x_slice(c, 0))
                nc.gpsimd.dma_start(*x_slice(c, 1))

            WARMUP_MMS = 8
            if WARMUP_MMS:
                wp = psum.tile([128, P // 2], F32, name="pt")
                for _ in range(WARMUP_MMS):
                    nc.tensor.matmul(
                        wp[:, 0:512],
                        lhsT=wz[:],
                        rhs=wx[:],
                        start=True,
                        stop=True,
                        perf_mode=mybir.MatmulPerfMode.DoubleRow,
                    )

            # Each candidate tile is processed as two half-width PSUM tiles
            # ([128, 1024] = 2 banks, bufs=4 = all 8 banks). A tile's PSUM
            # release op (ACT 1965ns / STT 2350ns at full width) exceeded
            # the 1728ns tile period, so with bufs=2 the PE stalled on
            # every release (measured 20us). At half width the release is
            # ~1.2-1.5us against a 2.6us three-buffer tolerance.
            #
            # Drain paths per half H = 2t+hf (engine balance vs the 110.6us
            # PE floor, measured op costs ACT 1110ns / STT 1278ns / TT
            # 688ns): V-halves (every 4th, 32x) use the fused DVE STT; the
            # rest use ACT h=psum+y2 then a DVE min. ACT ~106us, DVE
            # ~106us. (GpSimd cannot run TENSOR_TENSOR on TRN2.)
            #
            # A-halves' min ops are emitted with one half of lag so a
            # V-half's fused STT can run the moment its matmuls stop
            # (min-accumulation commutes, so chain order is free): the STT
            # would otherwise wait on ACT(H-1)+min(H-1) and free its PSUM
            # late.
            pending_min = []

            def flush_mins():
                while pending_min:
                    eng, acc_, h_ = pending_min.pop(0)
                    eng.tensor_tensor(
                        out=acc_, in0=acc_, in1=h_, op=mybir.AluOpType.min
                    )

            HP = P // 2  # half width
            for t in range(R_TILES):
                g, o = t // 4, (t % 4) * 128
                bias = y2t_sb[:, t : t + 1]
                for hf in range(2):
                    H = 2 * t + hf
                    acc_h = acc[:, hf * HP : (hf + 1) * HP]
                    pt = psum.tile([128, HP], F32, name="pt")
                    for c in (2 * hf, 2 * hf + 1):
                        for kk in range(K_TILES // 2):
                            nc.tensor.matmul(
                                pt[:, (c - 2 * hf) * 512 : (c - 2 * hf + 1) * 512],
                                lhsT=yt_sb[:, g, 2 * kk : 2 * kk + 2, o : o + 128],
                                rhs=xt_sb[:, c, 2 * kk : 2 * kk + 2, :],
                                start=(kk == 0),
                                stop=(kk == K_TILES // 2 - 1),
                                perf_mode=mybir.MatmulPerfMode.DoubleRow,
                            )
                    if H < 2:
                        # First drain of each acc column half initializes it.
                        nc.scalar.activation(
                            out=acc_h,
                            in_=pt[:],
                            func=mybir.ActivationFunctionType.Identity,
                            bias=bias,
                            scale=1.0,
                        )
                    elif H % 4 == 2 or H >= 126:
                        # Fused drain: acc = min(psum + y2, acc) in one DVE op.
                        nc.vector.scalar_tensor_tensor(
                            out=acc_h,
                            in0=pt[:],
                            scalar=bias,
                            in1=acc_h,
                            op0=mybir.AluOpType.add,
                            op1=mybir.AluOpType.min,
                        )
                        flush_mins()
                    else:
                        h = scr.tile([128, HP], BF16, name="h")
                        nc.scalar.activation(
                            out=h[:],
                            in_=pt[:],
                            func=mybir.ActivationFunctionType.Identity,
                            bias=bias,
                            scale=1.0,
                        )
                        flush_mins()
                        pending_min.append((nc.vector, acc_h, h[:]))
            flush_mins()
            # Ship raw on the two HWDGE rings only. GpSimd must stay idle
            # here: any late SWDGE DMA makes the end-of-kernel gp drain
            # cost ~5-7us (it is ~0.1us when gp has been quiet), and the
            # [128,1]-style reduced outputs crawl ~7us on HWDGE descriptors
            # — raw 128KB column slices are the fastest exit.
            nc.sync.dma_start(out.ap()[:, 0:512], acc[:, 0:512])
            nc.scalar.dma_start(out.ap()[:, 512:1024], acc[:, 512:1024])
            nc.sync.dma_start(out.ap()[:, 1024:1536], acc[:, 1024:1536])
            nc.scalar.dma_start(out.ap()[:, 1536:P], acc[:, 1536:P])
    nc.compile()
    return nc


_module_cache: bass.Bass | None = None


def _get_module() -> bass.Bass:
    global _module_cache
    if _module_cache is None:
        _module_cache = _build_module()
    return _module_cache


def _to_partition_major(at: np.ndarray, nchunks: int) -> np.ndarray:
    """[D, W] transposed operand -> [128, nchunks, K_TILES, 512] fp8."""
    w = at.shape[1]
    a4 = at.reshape(K_TILES, 128, nchunks, w // nchunks)
    return np.ascontiguousarray(a4.transpose(1, 2, 0, 3).astype(MM_NP))


def _prepare_inputs(x: np.ndarray, y: np.ndarray):
    """Host-side sharding/layout prep. Returns per-core input maps."""
    xt = _to_partition_major(x.T, P_CHUNKS)
    in_maps = []
    for c in range(NCORES):
        yc = y[c * R_LOC : (c + 1) * R_LOC]
        # -2 folded into the stationary operand: exact in fp8 (sign+exponent)
        yct = _to_partition_major(np.float32(-2.0) * yc.T, R_GROUPS)
        y2 = np.einsum("rd,rd->r", yc, yc, dtype=np.float32) - Y2_SHIFT
        y2t = np.ascontiguousarray(y2.reshape(R_TILES, 128).T)
        in_maps.append({"xt": xt, "yt": yct, "y2t": y2t})
    return in_maps


def _postprocess(x: np.ndarray, accs: np.ndarray) -> np.ndarray:
    """accs: [NCORES, 128, P] partial mins (missing the x2 term)."""
    m = accs.astype(np.float32).min(axis=(0, 1)) + Y2_SHIFT
    x2 = np.einsum("pd,pd->p", x, x, dtype=np.float32)
    sq_min = np.float32((x2 + m).min())
    return np.sqrt(np.maximum(sq_min, np.float32(0.0)), dtype=np.float32)


def kernel(
    predicted_transaction_company: np.ndarray,
    future_transaction_companies_inc_current_data: np.ndarray,
) -> np.ndarray:
    x = np.asarray(predicted_transaction_company, dtype=np.float32)[0]
    y = np.asarray(future_transaction_companies_inc_current_data, dtype=np.float32)[0]

    nc = _get_module()
    in_maps = _prepare_inputs(x, y)
    res = bass_utils.run_bass_kernel_spmd(nc, in_maps, core_ids=list(range(NCORES)))
    accs = np.stack([r["out"] for r in res.results])
    return _postprocess(x, accs)
